# revision 32
# baseline (speedup 1.0000x reference)
"""Trainium2 Bass kernel for nn_AttentionBlock (B=4, L=2048, C=512, H=8, Dk=64).

Sharding (8 cores): data-parallel over B (4) x tensor-parallel over heads (2
groups of 4). Core c handles batch c//2, head group c%2. Each core computes
  y_c = attention(x_b)[:, local_heads] @ W_out[local_rows]        [2048, 512]
and the host combines: out[b] = y[2b] + y[2b+1] + b_out + x[b].

Device kernel (per core); matmul operands bf16, fp32 PSUM accumulation:
  - qT/kT per head in [Dk, L] layout straight out of the projection
    (lhsT=W_in chunk, rhs=xT chunk) -- no transposes anywhere.
  - v in natural [L, Dk] layout, augmented with a ones column (plus zero
    padding to 128 for fast weight load) so the O^T = V^T P^T matmul also
    produces the softmax denominators for free.
  - scores S^T [keys, queries]; causal structure skips upper-triangle
    tiles and narrows diagonal-straddling tiles; the diagonal 128x128
    gets a precomputed 0/1 triangle multiply after exp.
  - exp batched over key-tile pairs (one 2-bank PSUM tile) to amortize
    ACT's 352-cycle per-instruction overhead.
  - softmax normalization off the PE queue: DVE copy + fast reciprocal,
    GpSimd partition-broadcast, DVE multiply.
  - emission order keeps the (strictly in-order) PE queue dense: a
    warmup burst covers the input-DMA wait, projections/out-projections
    are fed one unit at a time between attention pairs, and input DMA
    issue is spread across three engine queues. This also holds the PE
    HAM clock-gate at 2.4 GHz.
Only MM_MODE="bf16" fits SBUF; measured absmax-relative error vs the
fp32 reference is ~3e-3 (dominated by bf16 operand rounding).
"""

import sys

sys.path.insert(0, "/opt/trn_rl_repo")

import numpy as np

import concourse.bacc as bacc
import concourse.bass as bass
import concourse.mybir as mybir
import concourse.tile as tile
from concourse.bass_utils import run_bass_kernel_spmd

# ---------------------------------------------------------------- constants
B, L, C = 4, 2048, 512
H, DK = 8, 64
HPC = 4  # heads per core
SCALE = DK**-0.5
N_CORES = 8
KC = C // 128  # 4 contraction chunks
LT = L // 128  # 16 row tiles
QB = L // 512  # 4 query blocks of 512

F32 = mybir.dt.float32
BF16 = mybir.dt.bfloat16

# matmul operand dtype: "bf16" (fast) or "fp32" (exact)
MM_MODE = "bf16"

# test hooks (grading path leaves these alone)
TRACE = False
LAST_RESULT = None

_CACHE = {}


def _np_mm_dtype():
    if MM_MODE == "bf16":
        import ml_dtypes

        return ml_dtypes.bfloat16
    return np.float32


def _build(mm_mode):
    mm = BF16 if mm_mode == "bf16" else F32
    nc = bacc.Bacc(None)

    xT = nc.declare_dram_parameter("xT", [C, L], mm, isOutput=False)
    w_in = nc.declare_dram_parameter("w_in", [C, HPC, 192], mm, isOutput=False)
    qkb = nc.declare_dram_parameter("qkb", [64, 8], F32, isOutput=False)
    vb = nc.declare_dram_parameter("vb", [HPC, DK + 1], F32, isOutput=False)
    w_out = nc.declare_dram_parameter("w_out", [HPC, DK, C], mm, isOutput=False)
    tri = nc.declare_dram_parameter("tri", [128, 128], mm, isOutput=False)
    y = nc.declare_dram_parameter("y", [L, C], F32, isOutput=True)

    with tile.TileContext(nc) as tc:
        with (
            tc.tile_pool(name="persist", bufs=1) as per,
            tc.tile_pool(name="work", bufs=2) as work,
            tc.tile_pool(name="psum", bufs=1, space="PSUM") as psum,
        ):
            # ---------------- loads
            xT_sb = [per.tile([128, L], mm, tag=f"xT{i}", name=f"xT{i}") for i in range(KC)]
            w_in_sb = [per.tile([128, HPC, 192], mm, tag=f"wi{i}", name=f"wi{i}") for i in range(KC)]
            w_out_sb = per.tile([DK, HPC, C], mm, tag="wo")
            tri_sb = per.tile([128, 128], mm, tag="tri")
            qkb_sb = per.tile([64, 8], F32, tag="qkb")
            vb_sb = per.tile([128, HPC, DK + 1], F32, tag="vb")

            # PE warmup: dependency-free dummy matmuls fill the input-DMA
            # wait and hold the HAM clock-gate warm before real work starts
            # (otherwise warm/cold entry is start-phase luck, ~+30us).
            warm = per.tile([128, 512], mm, tag="warm")
            nc.vector.memset(warm, 0.0)
            wps = psum.tile([128, 512], F32, tag="ot", bufs=2, name="warmps")
            for _ in range(24):
                nc.tensor.matmul(
                    wps, lhsT=warm[:, 0:128], rhs=warm, start=True, stop=True
                )

            # Input loads: DMA issue is ~0.7us per dma_start per engine
            # queue, so spread the issues across four engine queues.
            xT_t = xT.rearrange("(c p) l -> c p l", p=128)
            w_in_t = w_in.rearrange("(c p) h d -> c p h d", p=128)
            for i in range(KC):
                eng = nc.sync if i < 2 else nc.scalar
                eng.dma_start(out=xT_sb[i][:, 0:1024], in_=xT_t[i][:, 0:1024])
                nc.gpsimd.dma_start(out=w_in_sb[i], in_=w_in_t[i])
            for i in range(KC):
                eng = nc.sync if i < 2 else nc.scalar
                eng.dma_start(out=xT_sb[i][:, 1024:L], in_=xT_t[i][:, 1024:L])
            nc.sync.dma_start(out=qkb_sb, in_=qkb[:, :])
            vb_ap = vb[:, :]
            vb_bcast = bass.AP(
                tensor=vb_ap.tensor, offset=vb_ap.offset, ap=[[0, 128], *vb_ap.ap]
            )
            nc.sync.dma_start(out=vb_sb, in_=vb_bcast)
            nc.sync.dma_start(out=tri_sb, in_=tri[:, :])
            nc.scalar.dma_start(out=w_out_sb, in_=w_out.rearrange("h d c -> d h c"))

            # ---------------- fused pipeline ----------------
            # Attention is ACT(exp)-bound per key-tile pair; projection and
            # out-projection matmuls are fed into the PE queue one unit at a
            # time between each pair's ST and OT matmuls so the PE fills its
            # exp-wait gaps with dense K=128 work (also keeps the HAM
            # clock-gate warm).
            qT_sb = [per.tile([DK, L], mm, tag=f"qT{h}", name=f"qT{h}") for h in range(HPC)]
            kT_sb = [per.tile([DK, L], mm, tag=f"kT{h}", name=f"kT{h}") for h in range(HPC)]
            v_sb = [per.tile([128, HPC, 128], mm, tag=f"v{lt}", name=f"v{lt}") for lt in range(LT)]
            ot_sb = [per.tile([DK, L], mm, tag=f"ot{h}", name=f"ot{h}") for h in range(HPC)]

            def emit_qk_unit(m, lc):
                h, half = divmod(m, 2)
                dst_t = qT_sb[h] if half == 0 else kT_sb[h]
                ps = psum.tile([DK, 512], F32, tag="mm", bufs=2, name="psqk")
                for kc in range(KC):
                    nc.tensor.matmul(
                        ps,
                        lhsT=w_in_sb[kc][:, h, 64 * half : 64 * half + 64],
                        rhs=xT_sb[kc][:, lc * 512 : (lc + 1) * 512],
                        start=(kc == 0),
                        stop=(kc == KC - 1),
                    )
                nc.vector.tensor_scalar_add(
                    dst_t[:, lc * 512 : (lc + 1) * 512], ps, qkb_sb[:, m : m + 1]
                )

            def emit_v_unit(lt):
                ps = psum.tile([128, HPC, DK], F32, tag="mm", bufs=2, name="psv")
                for kc in range(KC):
                    nc.tensor.matmul(
                        ps,
                        lhsT=xT_sb[kc][:, lt * 128 : (lt + 1) * 128],
                        rhs=w_in_sb[kc][:, :, 128:192],
                        start=(kc == 0),
                        stop=(kc == KC - 1),
                    )
                nc.vector.memset(v_sb[lt][:, :, DK:128], 0.0)
                nc.vector.tensor_add(v_sb[lt][:, :, 0:DK], ps, vb_sb[:, :, 0:DK])
                nc.vector.memset(v_sb[lt][:, :, DK : DK + 1], 1.0)

            def emit_outproj_unit(lt):
                yp = psum.tile([128, C], F32, tag="mm", bufs=2, name="psy")
                for h in range(HPC):
                    nc.tensor.matmul(
                        yp,
                        lhsT=ot_sb[h][:, lt * 128 : (lt + 1) * 128],
                        rhs=w_out_sb[:, h, :],
                        start=(h == 0),
                        stop=(h == HPC - 1),
                    )
                ysb = work.tile([128, C], F32, tag="ysb", bufs=3, name="ysb")
                nc.vector.tensor_copy(ysb, yp)
                nc.sync.dma_start(out=y[lt * 128 : (lt + 1) * 128, :], in_=ysb)

            def proj_units(lc):
                u = [(emit_qk_unit, (m, lc)) for m in range(2 * HPC)]
                u += [(emit_v_unit, (lt,)) for lt in range(4 * lc, 4 * lc + 4)]
                return u

            def emit_attention(qb, feed):
                nkj = 4 * qb + 4
                npairs = nkj // 2

                def st_exp(h, p):
                    st2 = psum.tile(
                        [128, 1024], F32, tag="st2", bufs=2, name="psst"
                    )
                    for half in range(2):
                        kj = 2 * p + half
                        r = kj - 4 * qb
                        # straddle tiles only feed queries at or beyond
                        # the diagonal: columns [128*r, 512)
                        ws = 128 * r if r > 0 else 0
                        nc.tensor.matmul(
                            st2[:, 512 * half + ws : 512 * (half + 1)],
                            lhsT=kT_sb[h][:, kj * 128 : (kj + 1) * 128],
                            rhs=qT_sb[h][:, qb * 512 + ws : (qb + 1) * 512],
                            start=True,
                            stop=True,
                        )
                    se = work.tile([128, 1024], mm, tag="se", bufs=4, name="se")
                    r0 = 2 * p - 4 * qb
                    ws0 = 128 * r0 if r0 > 0 else 0
                    nc.scalar.activation(
                        out=se[:, ws0:1024],
                        in_=st2[:, ws0:1024],
                        func=mybir.ActivationFunctionType.Exp,
                        scale=float(SCALE),
                    )
                    return se

                def mask_ot(h, p, se, ot):
                    for half in range(2):
                        kj = 2 * p + half
                        r = kj - 4 * qb
                        ws = 128 * r if r > 0 else 0
                        o = 512 * half
                        if r >= 0:
                            nc.vector.tensor_mul(
                                se[:, o + ws : o + ws + 128],
                                se[:, o + ws : o + ws + 128],
                                tri_sb,
                            )
                        nc.tensor.matmul(
                            ot[:, ws:512],
                            lhsT=v_sb[kj][:, h, :],
                            rhs=se[:, o + ws : o + 512],
                            start=(kj == 0),
                            stop=(kj == nkj - 1),
                        )

                def epilogue(h, ot):
                    # normalize: ot[:DK] /= ot[DK] -- all off the PE queue:
                    # DVE copy + fast reciprocal, GpSimd partition broadcast,
                    # DVE multiply.
                    dnm = work.tile([1, 512], F32, tag="dnm", bufs=2, name="dnm")
                    nc.vector.tensor_copy(dnm, ot[DK : DK + 1, :])
                    rcp = work.tile([1, 512], F32, tag="rcp", bufs=2, name="rcp")
                    nc.vector.reciprocal_approx_fast(out=rcp, in_=dnm)
                    rbs = work.tile([DK, 512], F32, tag="rbs", bufs=2, name="rbs")
                    nc.gpsimd.partition_broadcast(rbs, rcp)
                    nc.vector.tensor_mul(
                        ot_sb[h][:, qb * 512 : (qb + 1) * 512], ot[0:DK, :], rbs
                    )

                for h in range(HPC):
                    ot = psum.tile([128, 512], F32, tag="ot", bufs=2, name="psot")
                    for p in range(npairs):
                        se = st_exp(h, p)
                        if feed:
                            fn, args = feed.pop(0)
                            fn(*args)
                        mask_ot(h, p, se, ot)
                    epilogue(h, ot)

            # prelude: slice-0 projections, then attention groups with the
            # next slice's projections + previous block's out-proj as feed
            for fn, args in proj_units(0):
                fn(*args)
            for qb in range(QB):
                feed = []
                if qb + 1 < QB:
                    feed += proj_units(qb + 1)
                if qb >= 1:
                    feed += [(emit_outproj_unit, (lt,)) for lt in range(4 * (qb - 1), 4 * qb)]
                emit_attention(qb, feed)
                for fn, args in feed:
                    fn(*args)
                feed.clear()
            # hold the clock-gate warm while the last head's epilogue
            # drains, so the final out-projection runs at 2.4 GHz
            for _ in range(10):
                nc.tensor.matmul(
                    wps, lhsT=warm[:, 0:128], rhs=warm, start=True, stop=True
                )
            for lt in range(4 * (QB - 1), 4 * QB):
                emit_outproj_unit(lt)

    nc.finalize()
    return nc


def _get_nc():
    if MM_MODE not in _CACHE:
        _CACHE[MM_MODE] = _build(MM_MODE)
    return _CACHE[MM_MODE]


def _make_tri():
    # [j, i] = 1 iff i >= j (key j attends-allowed for query i)
    return np.triu(np.ones((128, 128), np.float32))


def kernel(x, W_in, b_in, W_out, b_out):
    x = np.asarray(x, np.float32)
    W_in = np.asarray(W_in, np.float32)
    b_in = np.asarray(b_in, np.float32)
    W_out = np.asarray(W_out, np.float32)
    b_out = np.asarray(b_out, np.float32)

    mmd = _np_mm_dtype()
    tri = _make_tri().astype(mmd)

    in_maps = []
    for c in range(N_CORES):
        b, j = divmod(c, 2)
        w_in_loc = W_in[:, j * 768 : (j + 1) * 768]  # [C, 768]
        b_in_loc = b_in[j * 768 : (j + 1) * 768]  # [768]
        xT = np.ascontiguousarray(x[b].T).astype(mmd)  # [C, L]
        w_in_3d = np.ascontiguousarray(w_in_loc.reshape(C, HPC, 192)).astype(mmd)
        qkb = np.empty((64, 8), np.float32)
        for m in range(8):
            h, half = divmod(m, 2)
            o = 192 * h + 64 * half
            qkb[:, m] = b_in_loc[o : o + 64]
        vb = np.zeros((HPC, DK + 1), np.float32)
        for h in range(HPC):
            vb[h, :DK] = b_in_loc[192 * h + 128 : 192 * h + 192]
        w_out_loc = np.empty((HPC, DK, C), np.float32)
        for h in range(HPC):
            hh = j * HPC + h
            w_out_loc[h] = W_out[hh * DK : (hh + 1) * DK, :]
        in_maps.append(
            dict(
                xT=xT,
                w_in=w_in_3d,
                qkb=qkb,
                vb=vb,
                w_out=w_out_loc.astype(mmd),
                tri=tri,
            )
        )

    nc = _get_nc()
    res = run_bass_kernel_spmd(
        nc, in_maps, core_ids=list(range(N_CORES)), trace=TRACE
    )
    global LAST_RESULT
    LAST_RESULT = res

    out = np.empty((B, L, C), np.float32)
    for b in range(B):
        out[b] = (
            res.results[2 * b]["y"]
            + res.results[2 * b + 1]["y"]
            + b_out[None, :]
            + x[b]
        )
    return out
